# revision 50
# baseline (speedup 1.0000x reference)
"""Multi-head attention (softmax over the QUERY axis) for Trainium2, 8 cores.

Reference (B=2, T=2048, E=1024, H=16, HD=64):
    q = X@Wq.T+bq ; k = ... ; v = ...   (per-head split)
    s = (q k^T)/sqrt(E), causal mask (key > query -> -inf)
    attn = softmax(s, axis=QUERY)  -> normalizes each key COLUMN over queries
    out  = attn @ v

Sharding: core c = (batch c//4, head-group c%4 of 4 heads = 2 duos).  No
collectives.

Math per core (d2=256 output dims, O stored [T, 256]):
  Projections run in fp8e4 DoubleRow with hi+lo error compensation:
  W' = 32*W split W8+Wl (fp8), X split X8+Xl; q32 ~= X8@W8 + Xl@W8 + X8@Wl
  (+32b).  Scores use bf16 Q/K: s_psum = 1024*s; exp applies
  scale=1/32768 and per-key bias -ln(c_k), c_k ~ sqrt(E[r_k]) (any c_k is
  mathematically exact: V-scaling self-corrects).  P~=P/c_k stored fp8
  (keys < 1792) / fp16 (last 256 keys); V~ = 32*v*c/r.  A@V runs fp8
  DoubleRow over PAIRS of key-tiles (contraction 256), P~ stationary and
  V~ moving with both heads side-by-side so every matmul outputs at
  partition 0 (walrus rejects DoubleRow dst partitions != 0); final
  PSUM->SBUF copy multiplies by 1/32.

Schedule: all P~/V~ stay SBUF-resident.  duo0 processes key-tiles
DESCENDING (small score tiles first -> exp starts ~6us in, gated only by
the first 1.5MB of DMA); duo1 ASCENDING so its own A@V q-tiles interleave
as soon as their key-pairs complete (short tail).  duo0's A@V q-tiles
inject into duo1's early iterations.  Proj chunks / V tiles / A@V emit as
"fillers" BETWEEN score-exp units so ACT (the bottleneck engine, ~89us
busy of ~109us span) never waits on the next scores matmul.
PSUM: 2x[128,1536] score slots + 2x[128,512] proj/V/A@V slots.
"""

import math
from contextlib import ExitStack

import numpy as np
import ml_dtypes

import concourse.bacc as bacc
import concourse.mybir as mybir
import concourse.tile as tile
from concourse.bass_utils import run_bass_kernel_spmd

B, T, E, H = 2, 2048, 1024, 16
D2 = 256              # output dims per core (4 heads)
NKT = 16              # key tiles of 128
NPAIR = 7             # fp8 DoubleRow pairs: kts (2p, 2p+1), p=0..6; kts 14,15 fp16
F32 = mybir.dt.float32
BF16 = mybir.dt.bfloat16
F16 = mybir.dt.float16
F8 = mybir.dt.float8e4
DRM = mybir.MatmulPerfMode.DoubleRow
EXP = mybir.ActivationFunctionType.Exp
AX = mybir.AxisListType.X
SCALE = 1.0 / 32768.0   # 1/sqrt(E) / 32 / 32  (both Q,K carry 32x weights)
NEG = -1.0e30
WS = 32.0               # weight prescale
np_f8 = ml_dtypes.float8_e4m3
np_bf16 = ml_dtypes.bfloat16

# w_t column-block offsets (per e-chunk, 1536 wide; Q/K are duo-major so
# the head only needs the first 512 W columns)
V8O, VLO = 1024, 1280

_CACHE = {}


def _build_module():
    nc = bacc.Bacc("TRN2", target_bir_lowering=False, debug=False)

    x8_d = nc.dram_tensor("x8", [128, 8, T], F8, kind="ExternalInput")
    xl_d = nc.dram_tensor("xl", [128, 8, T], F8, kind="ExternalInput")
    wqk_d = nc.dram_tensor("wqk", [128, 8, 1024], F8, kind="ExternalInput")
    wv8l_d = nc.dram_tensor("wv8l", [128, 8, 512], F8, kind="ExternalInput")
    # merged consts: fewer DMA instructions (each costs ~625ns of HWDGE)
    cf_d = nc.dram_tensor("cf", [128, NKT + 4], F32, kind="ExternalInput")
    mi_d = nc.dram_tensor("mi", [128, 256], BF16, kind="ExternalInput")
    ob_d = nc.dram_tensor("ob", [1, 128 + D2], BF16, kind="ExternalInput")
    ot_d = nc.dram_tensor("ot", [T, D2], F32, kind="ExternalOutput")

    with tile.TileContext(nc) as tc:
        _body(tc, x8_d, xl_d, wqk_d, wv8l_d, cf_d, mi_d, ob_d, ot_d)
    nc.compile()
    return nc


def _body(tc, x8_d, xl_d, wqk_d, wv8l_d, cf_d, mi_d, ob_d, ot_d):
    nc = tc.nc

    with ExitStack() as ctx:
        cp = ctx.enter_context(tc.tile_pool(name="const", bufs=1))
        cf_t = cp.tile([128, NKT + 4], F32)     # [biasc | bqc | bkc]
        mi_t = cp.tile([128, 256], BF16)        # [masku | ident]
        ob_t = cp.tile([1, 128 + D2], BF16)     # [ones | bvr]
        biasc_t = cf_t[:, 0:NKT]
        bqc_t = cf_t[:, NKT:NKT + 2]
        bkc_t = cf_t[:, NKT + 2:NKT + 4]
        masku_t = mi_t[:, 0:128]
        ident_t = mi_t[:, 128:256]
        ones_t = ob_t[:, 0:128]
        bvr_t = ob_t[:, 128:128 + D2]
        nc.sync.dma_start(ob_t[:], ob_d.ap())

        xw = ctx.enter_context(tc.tile_pool(name="xw", bufs=1))
        x8_t = xw.tile([128, 8, T], F8)
        xl_t = xw.tile([128, 8, T], F8)
        w_t = xw.tile([128, 8, 1536], F8)

        qk = ctx.enter_context(tc.tile_pool(name="qk", bufs=1))
        qt_t = qk.tile([128, 2, T], BF16)
        kt_t = qk.tile([128, 2, T], BF16)

        vtp = ctx.enter_context(tc.tile_pool(name="vt", bufs=1))
        v_t = vtp.tile([128, NKT, D2], BF16)

        pp = ctx.enter_context(tc.tile_pool(name="pp", bufs=1))
        vp = ctx.enter_context(tc.tile_pool(name="vp", bufs=1))
        st = ctx.enter_context(tc.tile_pool(name="st", bufs=6))
        osb = ctx.enter_context(tc.tile_pool(name="osb", bufs=1))

        sc_pool = ctx.enter_context(
            tc.tile_pool(name="scp", bufs=2, space="PSUM"))
        pv_pool = ctx.enter_context(
            tc.tile_pool(name="pvp", bufs=1, space="PSUM"))

        # ---- input DMA, ordered for earliest exp start (pieces stay
        # 512-col wide: narrower rows pay a 2x DMA descriptor penalty) ----
        nc.sync.dma_start(w_t[:, :, 0:512], wqk_d.ap()[:, :, 0:512])
        nc.sync.dma_start(x8_t[:, :, 1536:2048], x8_d.ap()[:, :, 1536:2048])
        nc.sync.dma_start(mi_t[:], mi_d.ap())
        nc.sync.dma_start(cf_t[:], cf_d.ap())
        nc.sync.dma_start(xl_t[:, :, 1536:2048], xl_d.ap()[:, :, 1536:2048])
        nc.sync.dma_start(w_t[:, :, 512:1024], wqk_d.ap()[:, :, 512:1024])
        nc.sync.dma_start(x8_t[:, :, 1024:1536], x8_d.ap()[:, :, 1024:1536])
        nc.sync.dma_start(xl_t[:, :, 1024:1536], xl_d.ap()[:, :, 1024:1536])
        nc.sync.dma_start(w_t[:, :, 1024:1536], wv8l_d.ap())
        for tb in (1, 0):
            nc.sync.dma_start(x8_t[:, :, tb * 512:tb * 512 + 512],
                              x8_d.ap()[:, :, tb * 512:tb * 512 + 512])
            nc.sync.dma_start(xl_t[:, :, tb * 512:tb * 512 + 512],
                              xl_d.ap()[:, :, tb * 512:tb * 512 + 512])

        # warm the exp table off the critical path
        warm_t = st.tile([1, 2], F32, name="warm")
        nc.scalar.activation(warm_t[:], ones_t[0:1, 0:2], EXP,
                             bias=0.0, scale=SCALE)

        # ---- P~ / V~ tiles (SBUF-resident until the duo's A@V) ----
        p_pair = {}   # (duo, hh, p) -> [128, 2, Wp] fp8
        p_tail = {}   # (duo, hh, kt in 14,15) -> [128, W] fp16
        vp_pair = {}  # (duo, p) -> [128, 2, 128] fp8   (both heads in free)
        vp_tail = {}  # (duo, kt) -> [128, 128] fp16
        for d in range(2):
            for hh in range(2):
                for p in range(NPAIR):
                    wp = T - 256 * p
                    p_pair[(d, hh, p)] = pp.tile(
                        [128, 2, wp], F8, tag=f"p{d}_{hh}_{p}",
                        name=f"p{d}_{hh}_{p}")
                for kt in (14, 15):
                    p_tail[(d, hh, kt)] = pp.tile(
                        [128, T - 128 * kt], F16, tag=f"pt{d}_{hh}_{kt}",
                        name=f"pt{d}_{hh}_{kt}")
            for kt in (14, 15):
                vp_tail[(d, kt)] = vp.tile(
                    [128, 128], F16, tag=f"vpt{d}_{kt}",
                    name=f"vpt{d}_{kt}")
            for p in range(NPAIR):
                vp_pair[(d, p)] = vp.tile(
                    [128, 2, 128], F8, tag=f"vp{d}_{p}",
                    name=f"vp{d}_{p}")
        # zero the odd-kt first-128 strips (masked region the exp never
        # writes); gpsimd keeps this off the busy engines
        for d in range(2):
            for hh in range(2):
                for p in range(NPAIR):
                    nc.gpsimd.memset(p_pair[(d, hh, p)][:, 1, 0:128], 0.0)

        # ---- PE helper emitters ----
        pv_tog = [0]

        def pv_tile(name):
            tag = ("pj", "ob")[pv_tog[0] % 2]
            pv_tog[0] += 1
            return pv_pool.tile([128, 512], F32, tag=tag, name=name)

        def emit_qk_chunk(duo, is_k, c, off=0, n=512, passes=3):
            # one chunk of the Q^T/K^T projection for `duo`:
            # psum = X8.T@W8 + Xl.T@W8 + X8.T@Wl  (12 fp8 DoubleRow matmuls).
            # passes=2 drops the Xl@W8 term (head chunks: run before the Xl
            # DMA lands; a full-precision patch re-projects them later)
            w8o = duo * 512 + (128 if is_k else 0)
            wlo = w8o + 256
            out_t, b_t = (kt_t, bkc_t) if is_k else (qt_t, bqc_t)
            c0 = c * 512 + off
            ps = pv_tile(f"qk{duo}_{int(is_k)}_{c}_{off}")
            first = True
            plan = ((x8_t, w8o), (x8_t, wlo), (xl_t, w8o))[:passes]
            for pidx, (xs, wo) in enumerate(plan):
                for ep in range(4):
                    nc.tensor.matmul(
                        ps[:, 0:n],
                        lhsT=w_t[:, 2 * ep:2 * ep + 2, wo:wo + 128],
                        rhs=xs[:, 2 * ep:2 * ep + 2, c0:c0 + n],
                        start=first,
                        stop=(pidx == len(plan) - 1 and ep == 3),
                        perf_mode=DRM,
                    )
                    first = False
            nc.vector.tensor_scalar_add(
                out_t[:, duo, c0:c0 + n], ps[:, 0:n],
                b_t[:, duo:duo + 1])

        def emit_v_tile(kt):
            # V tile (both duos): [128 t, 256 d] = X.T@Wv*32 + 32*bv
            ps = pv_tile(f"v{kt}")
            pvs = ps[:, 0:D2]
            for xs, wo in ((x8_t, V8O), (xl_t, V8O), (x8_t, VLO)):
                for ep in range(4):
                    nc.tensor.matmul(
                        pvs,
                        lhsT=xs[:, 2 * ep:2 * ep + 2,
                                kt * 128:kt * 128 + 128],
                        rhs=w_t[:, 2 * ep:2 * ep + 2, wo:wo + D2],
                        start=(xs is x8_t and wo == V8O and ep == 0),
                        stop=False,
                        perf_mode=DRM,
                    )
            nc.tensor.matmul(pvs, lhsT=ones_t[0:1, :], rhs=bvr_t[0:1, :],
                             start=False, stop=True)
            nc.vector.tensor_copy(v_t[:, kt, :], pvs)

        def emit_scores_exp(duo, kt, fillers=()):
            # scores S^T[key, q] for q in [qlo, T), exp'd into P~ with
            # per-key bias -ln(c_k); accum -> rs (per-key sums r~).
            # `fillers`: PE work emitted between score/exp units so the
            # engine pipeline never leaves ACT waiting on the next scores.
            fillers = list(fillers)
            qlo = kt * 128
            w = T - qlo
            pieces = [(0, min(w, 1536))]
            if w > 1536:
                pieces.append((1536, w - 1536))
            rs_t = st.tile([128, 2], F32, tag="rs", name=f"rs{duo}_{kt}")
            sums_t = (st.tile([128, 4], F32, tag="sums", name=f"sm{duo}_{kt}")
                      if len(pieces) > 1 else None)
            for hh in range(2):
                d0 = 64 * hh
                for pi, (poff, pw) in enumerate(pieces):
                    if hh + pi > 0 and fillers:
                        fillers.pop(0)()
                    sc = sc_pool.tile([128, 1536], F32, tag="sc", name="sc")
                    for co in range(0, pw, 512):
                        n = min(512, pw - co)
                        nc.tensor.matmul(
                            sc[:, co:co + n],
                            lhsT=kt_t[d0:d0 + 64, duo, qlo:qlo + 128],
                            rhs=qt_t[d0:d0 + 64, duo,
                                     qlo + poff + co:qlo + poff + co + n],
                            start=True,
                            stop=not (poff == 0 and co == 0),
                        )
                    if poff == 0:
                        nc.tensor.matmul(
                            sc[:, 0:128], lhsT=masku_t[:, 0:128],
                            rhs=ident_t[:], start=False, stop=True,
                            skip_group_check=True)
                    if kt >= 14:
                        dst = p_tail[(duo, hh, kt)][:, poff:poff + pw]
                    else:
                        p = kt // 2
                        par = kt % 2
                        off = 128 * par + poff
                        dst = p_pair[(duo, hh, p)][:, par, off:off + pw]
                    acc = (sums_t[:, hh * 2 + pi:hh * 2 + pi + 1]
                           if sums_t is not None else rs_t[:, hh:hh + 1])
                    nc.scalar.activation(
                        dst, sc[:, 0:pw], EXP,
                        bias=biasc_t[:, kt:kt + 1], scale=SCALE,
                        accum_out=acc)
            if sums_t is not None:
                for hh in range(2):
                    nc.vector.reduce_sum(
                        rs_t[:, hh:hh + 1], sums_t[:, hh * 2:hh * 2 + 2],
                        axis=AX)
            # run all but one leftover now; carry the last to the next
            # iteration so a filler burst never sits between two kts'
            # scores (order preserved: FIFO across the boundary)
            for f in fillers[:-1]:
                f()
            return rs_t, fillers[-1:]

        def emit_vtilde(duo, kt, rs_t):
            # rinv = 1/r~ ; V~ = 32*v*rinv (fp8 pairs / fp16 tail)
            rinv_t = st.tile([128, 2], F32, tag="rinv", name=f"ri{duo}_{kt}")
            nc.vector.reciprocal(rinv_t[:], rs_t[:])
            for hh in range(2):
                if kt >= 14:
                    dst = vp_tail[(duo, kt)][:, 64 * hh:64 * hh + 64]
                else:
                    dst = vp_pair[(duo, kt // 2)][:, kt % 2,
                                                  64 * hh:64 * hh + 64]
                nc.vector.tensor_scalar_mul(
                    dst,
                    v_t[:, kt, duo * 128 + 64 * hh:duo * 128 + 64 * hh + 64],
                    rinv_t[:, hh:hh + 1])

        def emit_av_qtile(duo, j, ot_sb):
            # O[q, d] for q-tile j: fp8 DoubleRow, P~ stationary, V~ moving
            # with both heads side-by-side; fp16 solo for key-tiles 14, 15.
            # Out partitions = queries (always base 0).
            ob = pv_tile(f"av{duo}_{j}")
            obq = ob[:, 0:128]
            plast = min(j // 2, NPAIR - 1)
            for hh in range(2):
                for p in range(plast + 1):
                    c0 = 128 * j - 256 * p
                    nc.tensor.matmul(
                        obq[:, 64 * hh:64 * hh + 64],
                        lhsT=p_pair[(duo, hh, p)][:, :, c0:c0 + 128],
                        rhs=vp_pair[(duo, p)][:, :, 64 * hh:64 * hh + 64],
                        start=(p == 0),
                        stop=(j < 14 and p == plast),
                        perf_mode=DRM,
                        skip_group_check=True,
                    )
                for kt in (14, 15):
                    if kt > j:
                        continue
                    c0 = 128 * j - 128 * kt
                    nc.tensor.matmul(
                        obq[:, 64 * hh:64 * hh + 64],
                        lhsT=p_tail[(duo, hh, kt)][:, c0:c0 + 128],
                        rhs=vp_tail[(duo, kt)][:, 64 * hh:64 * hh + 64],
                        start=False,
                        stop=(kt == min(j, 15)),
                        skip_group_check=True,
                    )
            nc.vector.tensor_scalar_mul(
                ot_sb[:, 128 * j:128 * j + 128], obq, 1.0 / 32.0)
            nc.sync.dma_start(
                ot_d.ap()[128 * j:128 * j + 128,
                          duo * 128:duo * 128 + 128],
                ot_sb[:, 128 * j:128 * j + 128])

        # ---- schedule ----
        # duo0 fillers: own Q/K chunks just-in-time, duo1 Q/K chunks and V
        # tiles pushed toward big-kt iterations (wider ACT windows)
        qk_sched = {13: [(0, False, 2)], 12: [(0, True, 2)],
                    9: [(0, False, 1)], 8: [(0, True, 1)],
                    7: [(1, False, 3)], 6: [(1, True, 3)],
                    5: [(0, False, 0)], 4: [(0, True, 0)],
                    3: [(1, False, 2)], 2: [(1, True, 2)],
                    1: [(1, False, 1), (1, True, 1)],
                    0: [(1, False, 0), (1, True, 0)]}

        # head: 256-col sub-chunks; the [1792:2048] halves unblock kt15/14,
        # the [1536:1792] halves are emitted BETWEEN kt15 and kt13 (PE runs
        # its queue in program order)
        emit_qk_chunk(0, False, 3, off=256, n=256)
        emit_qk_chunk(0, True, 3, off=256, n=256)
        head_sched = {15: [(0, False, 3, 0, 256)],
                      14: [(0, True, 3, 0, 256)]}
        qk_sched = {k: head_sched.get(k, []) + qk_sched.get(k, [])
                    for k in set(head_sched) | set(qk_sched)}

        ot_sbs = [osb.tile([128, T], F32, tag=f"osb{d}", name=f"osb{d}")
                  for d in range(2)]
        # duo0's 16 A@V q-tiles, injected early into duo1's phase A
        av0_sched = {0: (0, 1), 1: (2, 3), 2: (4, 5), 3: (6, 7),
                     4: (8, 9), 5: (10, 11), 6: (12, 13), 7: (14, 15)}

        # duo0: key-tiles DESCENDING (small score tiles first -> early exp
        # start under partial DMA); duo1: ASCENDING so its own A@V q-tiles
        # interleave as soon as their key-pairs complete (short tail).
        rs_pend = {}
        carry = []
        for kt in range(NKT - 1, -1, -1):
            fills = []
            for args in qk_sched.get(kt, ()):
                fills.append(lambda a=args: emit_qk_chunk(*a))

            def vfill(jj):
                emit_v_tile(jj)
                emit_vtilde(0, jj, rs_pend.pop(jj))
            jj = kt + 4
            if jj <= NKT - 1:
                fills.append(lambda j=jj: vfill(j))
            if kt == 0:
                for jj in (3, 2, 1):
                    fills.append(lambda j=jj: vfill(j))
            rs_pend[kt], carry = emit_scores_exp(0, kt, carry + fills)
        for f in carry:
            f()
        carry = []
        vfill(0)
        rs_prev = None
        for kt in range(NKT):
            fills = []
            if kt >= 1:
                pk = kt - 1
                fills.append(
                    lambda p=pk, r=rs_prev: emit_vtilde(1, p, r))
                if pk % 2 == 1 and pk <= 13:
                    fills.append(
                        lambda p=pk: emit_av_qtile(1, p - 1, ot_sbs[1]))
                    fills.append(
                        lambda p=pk: emit_av_qtile(1, p, ot_sbs[1]))
                elif pk == 14:
                    fills.append(
                        lambda: emit_av_qtile(1, 14, ot_sbs[1]))
            for j in av0_sched.get(kt, ()):
                fills.append(lambda jj=j: emit_av_qtile(0, jj, ot_sbs[0]))
            rs_prev, carry = emit_scores_exp(1, kt, carry + fills)
        for f in carry:
            f()
        emit_vtilde(1, 15, rs_prev)
        emit_av_qtile(1, 15, ot_sbs[1])


def _get_module():
    if "nc" not in _CACHE:
        _CACHE["nc"] = _build_module()
    return _CACHE["nc"]


def _host_tables():
    k = np.arange(T)
    c = np.where(
        k < T - 256,
        2.0 ** np.round(0.5 * np.log2(1.031 * (T - k))),
        1.0)
    biasc = (-np.log(c)).reshape(NKT, 128).T.astype(np.float32)
    qi = np.arange(128)
    masku = np.where(qi[:, None] < qi[None, :], NEG, 0.0).astype(np_bf16)
    ident = np.eye(128, dtype=np.float32).astype(np_bf16)
    ones = np.ones((1, 128), np.float32).astype(np_bf16)
    return biasc, masku, ident, ones


def _split8(a):
    hi = a.astype(np_f8)
    lo = (a - hi.astype(np.float32)).astype(np_f8)
    return hi, lo


def _make_in_maps(X, Wq, bq, Wk, bk, Wv, bv):
    X = np.asarray(X, np.float32)
    biasc, masku, ident, ones = _host_tables()
    in_maps = []
    for core in range(8):
        b, g = divmod(core, 4)
        rows = slice(D2 * g, D2 * g + D2)
        xt = np.ascontiguousarray(X[b].T)              # [E, T]
        x8, xl = _split8(xt)
        x8 = np.ascontiguousarray(x8.reshape(8, 128, T).transpose(1, 0, 2))
        xl = np.ascontiguousarray(xl.reshape(8, 128, T).transpose(1, 0, 2))

        def wprep(Wfull):
            ws = np.asarray(Wfull)[rows].T.astype(np.float32) * WS  # [E, 256]
            return _split8(ws)

        wq8, wql = wprep(Wq)
        wk8, wkl = wprep(Wk)
        wv8, wvl = wprep(Wv)
        wqk = np.concatenate(
            [wq8[:, :128], wk8[:, :128], wql[:, :128], wkl[:, :128],
             wq8[:, 128:], wk8[:, 128:], wql[:, 128:], wkl[:, 128:]],
            axis=1)  # [E, 1024], duo-major
        wv8l = np.concatenate([wv8, wvl], axis=1)

        def dr3(w):  # [E, n] -> [128, 8, n]
            n = w.shape[1]
            return np.ascontiguousarray(
                w.reshape(8, 128, n).transpose(1, 0, 2))

        bqc = (WS * np.asarray(bq)[rows]).reshape(2, 128).T
        bkc = (WS * np.asarray(bk)[rows]).reshape(2, 128).T
        cf = np.concatenate([biasc, bqc, bkc], axis=1).astype(np.float32)
        mi = np.concatenate([masku, ident], axis=1)
        ob = np.concatenate(
            [ones, (WS * np.asarray(bv)[rows]).reshape(1, D2).astype(np_bf16)],
            axis=1)
        in_maps.append({
            "x8": x8, "xl": xl,
            "wqk": dr3(wqk), "wv8l": dr3(wv8l),
            "cf": np.ascontiguousarray(cf),
            "mi": np.ascontiguousarray(mi),
            "ob": np.ascontiguousarray(ob),
        })
    return in_maps


def kernel(X, Wq, bq, Wk, bk, Wv, bv, **kw):
    in_maps = _make_in_maps(X, Wq, bq, Wk, bk, Wv, bv)
    nc = _get_module()
    res = run_bass_kernel_spmd(nc, in_maps, core_ids=list(range(8)), **kw)
    _CACHE["last_res"] = res
    out = np.zeros((B, T, E), np.float32)
    for c in range(8):
        b, g = divmod(c, 4)
        out[b, :, D2 * g:D2 * g + D2] = res.results[c]["ot"]
    return out


if __name__ == "__main__":
    _get_module()
    print("module built ok")


# revision 51
# speedup vs baseline: 1.0022x; 1.0022x over previous
"""Multi-head attention (softmax over the QUERY axis) for Trainium2, 8 cores.

Reference (B=2, T=2048, E=1024, H=16, HD=64):
    q = X@Wq.T+bq ; k = ... ; v = ...   (per-head split)
    s = (q k^T)/sqrt(E), causal mask (key > query -> -inf)
    attn = softmax(s, axis=QUERY)  -> normalizes each key COLUMN over queries
    out  = attn @ v

Sharding: core c = (batch c//4, head-group c%4 of 4 heads = 2 duos).  No
collectives.

Math per core (d2=256 output dims, O stored [T, 256]):
  Projections run in fp8e4 DoubleRow with hi+lo error compensation:
  W' = 32*W split W8+Wl (fp8), X split X8+Xl; q32 ~= X8@W8 + Xl@W8 + X8@Wl
  (+32b).  Scores use bf16 Q/K: s_psum = 1024*s; exp applies
  scale=1/32768 and per-key bias -ln(c_k), c_k ~ sqrt(E[r_k]) (any c_k is
  mathematically exact: V-scaling self-corrects).  P~=P/c_k stored fp8
  (keys < 1792) / fp16 (last 256 keys); V~ = 32*v*c/r.  A@V runs fp8
  DoubleRow over PAIRS of key-tiles (contraction 256), P~ stationary and
  V~ moving with both heads side-by-side so every matmul outputs at
  partition 0 (walrus rejects DoubleRow dst partitions != 0); final
  PSUM->SBUF copy multiplies by 1/32.

Schedule: all P~/V~ stay SBUF-resident.  duo0 processes key-tiles
DESCENDING (small score tiles first -> exp starts ~6us in, gated only by
the first 1.5MB of DMA); duo1 ASCENDING so its own A@V q-tiles interleave
as soon as their key-pairs complete (short tail).  duo0's A@V q-tiles
inject into duo1's early iterations.  Proj chunks / V tiles / A@V emit as
"fillers" BETWEEN score-exp units so ACT (the bottleneck engine, ~89us
busy of ~109us span) never waits on the next scores matmul.
PSUM: 2x[128,1536] score slots + 2x[128,512] proj/V/A@V slots.
"""

import math
from contextlib import ExitStack

import numpy as np
import ml_dtypes

import concourse.bacc as bacc
import concourse.mybir as mybir
import concourse.tile as tile
from concourse.bass_utils import run_bass_kernel_spmd

B, T, E, H = 2, 2048, 1024, 16
D2 = 256              # output dims per core (4 heads)
NKT = 16              # key tiles of 128
NPAIR = 7             # fp8 DoubleRow pairs: kts (2p, 2p+1), p=0..6; kts 14,15 fp16
F32 = mybir.dt.float32
BF16 = mybir.dt.bfloat16
F16 = mybir.dt.float16
F8 = mybir.dt.float8e4
DRM = mybir.MatmulPerfMode.DoubleRow
EXP = mybir.ActivationFunctionType.Exp
AX = mybir.AxisListType.X
SCALE = 1.0 / 32768.0   # 1/sqrt(E) / 32 / 32  (both Q,K carry 32x weights)
NEG = -1.0e30
WS = 32.0               # weight prescale
np_f8 = ml_dtypes.float8_e4m3
np_bf16 = ml_dtypes.bfloat16

# w_t column-block offsets (per e-chunk, 1536 wide; Q/K are duo-major so
# the head only needs the first 512 W columns)
V8O, VLO = 1024, 1280

_CACHE = {}


def _build_module():
    nc = bacc.Bacc("TRN2", target_bir_lowering=False, debug=False)

    x8_d = nc.dram_tensor("x8", [128, 8, T], F8, kind="ExternalInput")
    xl_d = nc.dram_tensor("xl", [128, 8, T], F8, kind="ExternalInput")
    wqk_d = nc.dram_tensor("wqk", [128, 8, 1024], F8, kind="ExternalInput")
    wv8l_d = nc.dram_tensor("wv8l", [128, 8, 512], F8, kind="ExternalInput")
    # merged consts: fewer DMA instructions (each costs ~625ns of HWDGE)
    cf_d = nc.dram_tensor("cf", [128, NKT + 4], F32, kind="ExternalInput")
    mi_d = nc.dram_tensor("mi", [128, 256], BF16, kind="ExternalInput")
    ob_d = nc.dram_tensor("ob", [1, 128 + D2], BF16, kind="ExternalInput")
    ot_d = nc.dram_tensor("ot", [T, D2], F32, kind="ExternalOutput")

    with tile.TileContext(nc) as tc:
        _body(tc, x8_d, xl_d, wqk_d, wv8l_d, cf_d, mi_d, ob_d, ot_d)
    nc.compile()
    return nc


def _body(tc, x8_d, xl_d, wqk_d, wv8l_d, cf_d, mi_d, ob_d, ot_d):
    nc = tc.nc

    with ExitStack() as ctx:
        cp = ctx.enter_context(tc.tile_pool(name="const", bufs=1))
        cf_t = cp.tile([128, NKT + 4], F32)     # [biasc | bqc | bkc]
        mi_t = cp.tile([128, 256], BF16)        # [masku | ident]
        ob_t = cp.tile([1, 128 + D2], BF16)     # [ones | bvr]
        biasc_t = cf_t[:, 0:NKT]
        bqc_t = cf_t[:, NKT:NKT + 2]
        bkc_t = cf_t[:, NKT + 2:NKT + 4]
        masku_t = mi_t[:, 0:128]
        ident_t = mi_t[:, 128:256]
        ones_t = ob_t[:, 0:128]
        bvr_t = ob_t[:, 128:128 + D2]
        nc.sync.dma_start(ob_t[:], ob_d.ap())

        xw = ctx.enter_context(tc.tile_pool(name="xw", bufs=1))
        x8_t = xw.tile([128, 8, T], F8)
        xl_t = xw.tile([128, 8, T], F8)
        w_t = xw.tile([128, 8, 1536], F8)

        qk = ctx.enter_context(tc.tile_pool(name="qk", bufs=1))
        qt_t = qk.tile([128, 2, T], BF16)
        kt_t = qk.tile([128, 2, T], BF16)

        vtp = ctx.enter_context(tc.tile_pool(name="vt", bufs=1))
        v_t = vtp.tile([128, NKT, D2], BF16)

        pp = ctx.enter_context(tc.tile_pool(name="pp", bufs=1))
        vp = ctx.enter_context(tc.tile_pool(name="vp", bufs=1))
        st = ctx.enter_context(tc.tile_pool(name="st", bufs=6))
        osb = ctx.enter_context(tc.tile_pool(name="osb", bufs=1))

        sc_pool = ctx.enter_context(
            tc.tile_pool(name="scp", bufs=2, space="PSUM"))
        pv_pool = ctx.enter_context(
            tc.tile_pool(name="pvp", bufs=1, space="PSUM"))

        # ---- input DMA, ordered for earliest exp start (pieces stay
        # 512-col wide: narrower rows pay a 2x DMA descriptor penalty) ----
        nc.sync.dma_start(w_t[:, :, 0:512], wqk_d.ap()[:, :, 0:512])
        nc.sync.dma_start(x8_t[:, :, 1536:2048], x8_d.ap()[:, :, 1536:2048])
        nc.sync.dma_start(mi_t[:], mi_d.ap())
        nc.sync.dma_start(cf_t[:], cf_d.ap())
        nc.sync.dma_start(xl_t[:, :, 1536:2048], xl_d.ap()[:, :, 1536:2048])
        nc.sync.dma_start(w_t[:, :, 512:1024], wqk_d.ap()[:, :, 512:1024])
        nc.sync.dma_start(x8_t[:, :, 1024:1536], x8_d.ap()[:, :, 1024:1536])
        nc.sync.dma_start(xl_t[:, :, 1024:1536], xl_d.ap()[:, :, 1024:1536])
        nc.sync.dma_start(w_t[:, :, 1024:1536], wv8l_d.ap())
        for tb in (1, 0):
            nc.sync.dma_start(x8_t[:, :, tb * 512:tb * 512 + 512],
                              x8_d.ap()[:, :, tb * 512:tb * 512 + 512])
            nc.sync.dma_start(xl_t[:, :, tb * 512:tb * 512 + 512],
                              xl_d.ap()[:, :, tb * 512:tb * 512 + 512])

        # warm the exp table off the critical path
        warm_t = st.tile([1, 2], F32, name="warm")
        nc.scalar.activation(warm_t[:], ones_t[0:1, 0:2], EXP,
                             bias=0.0, scale=SCALE)

        # ---- P~ / V~ tiles (SBUF-resident until the duo's A@V) ----
        p_pair = {}   # (duo, hh, p) -> [128, 2, Wp] fp8
        p_tail = {}   # (duo, hh, kt in 14,15) -> [128, W] fp16
        vp_pair = {}  # (duo, p) -> [128, 2, 128] fp8   (both heads in free)
        vp_tail = {}  # (duo, kt) -> [128, 128] fp16
        for d in range(2):
            for hh in range(2):
                for p in range(NPAIR):
                    wp = T - 256 * p
                    p_pair[(d, hh, p)] = pp.tile(
                        [128, 2, wp], F8, tag=f"p{d}_{hh}_{p}",
                        name=f"p{d}_{hh}_{p}")
                for kt in (14, 15):
                    p_tail[(d, hh, kt)] = pp.tile(
                        [128, T - 128 * kt], F16, tag=f"pt{d}_{hh}_{kt}",
                        name=f"pt{d}_{hh}_{kt}")
            for kt in (14, 15):
                vp_tail[(d, kt)] = vp.tile(
                    [128, 128], F16, tag=f"vpt{d}_{kt}",
                    name=f"vpt{d}_{kt}")
            for p in range(NPAIR):
                vp_pair[(d, p)] = vp.tile(
                    [128, 2, 128], F8, tag=f"vp{d}_{p}",
                    name=f"vp{d}_{p}")
        # zero the odd-kt first-128 strips (masked region the exp never
        # writes); gpsimd keeps this off the busy engines
        for d in range(2):
            for hh in range(2):
                for p in range(NPAIR):
                    nc.gpsimd.memset(p_pair[(d, hh, p)][:, 1, 0:128], 0.0)

        # ---- PE helper emitters ----
        pv_tog = [0]

        def pv_tile(name):
            tag = ("pj", "ob")[pv_tog[0] % 2]
            pv_tog[0] += 1
            return pv_pool.tile([128, 512], F32, tag=tag, name=name)

        def emit_qk_chunk(duo, is_k, c, off=0, n=512, passes=3):
            # one chunk of the Q^T/K^T projection for `duo`:
            # psum = X8.T@W8 + Xl.T@W8 + X8.T@Wl  (12 fp8 DoubleRow matmuls).
            # passes=2 drops the Xl@W8 term (head chunks: run before the Xl
            # DMA lands; a full-precision patch re-projects them later)
            w8o = duo * 512 + (128 if is_k else 0)
            wlo = w8o + 256
            out_t, b_t = (kt_t, bkc_t) if is_k else (qt_t, bqc_t)
            c0 = c * 512 + off
            ps = pv_tile(f"qk{duo}_{int(is_k)}_{c}_{off}")
            first = True
            plan = ((x8_t, w8o), (x8_t, wlo), (xl_t, w8o))[:passes]
            for pidx, (xs, wo) in enumerate(plan):
                for ep in range(4):
                    nc.tensor.matmul(
                        ps[:, 0:n],
                        lhsT=w_t[:, 2 * ep:2 * ep + 2, wo:wo + 128],
                        rhs=xs[:, 2 * ep:2 * ep + 2, c0:c0 + n],
                        start=first,
                        stop=(pidx == len(plan) - 1 and ep == 3),
                        perf_mode=DRM,
                    )
                    first = False
            nc.vector.tensor_scalar_add(
                out_t[:, duo, c0:c0 + n], ps[:, 0:n],
                b_t[:, duo:duo + 1])

        def emit_v_tile(kt):
            # V tile (both duos): [128 t, 256 d] = X.T@Wv*32 + 32*bv
            ps = pv_tile(f"v{kt}")
            pvs = ps[:, 0:D2]
            for xs, wo in ((x8_t, V8O), (xl_t, V8O), (x8_t, VLO)):
                for ep in range(4):
                    nc.tensor.matmul(
                        pvs,
                        lhsT=xs[:, 2 * ep:2 * ep + 2,
                                kt * 128:kt * 128 + 128],
                        rhs=w_t[:, 2 * ep:2 * ep + 2, wo:wo + D2],
                        start=(xs is x8_t and wo == V8O and ep == 0),
                        stop=False,
                        perf_mode=DRM,
                    )
            nc.tensor.matmul(pvs, lhsT=ones_t[0:1, :], rhs=bvr_t[0:1, :],
                             start=False, stop=True)
            nc.vector.tensor_copy(v_t[:, kt, :], pvs)

        def emit_scores_exp(duo, kt, fillers=()):
            # scores S^T[key, q] for q in [qlo, T), exp'd into P~ with
            # per-key bias -ln(c_k); accum -> rs (per-key sums r~).
            # `fillers`: PE work emitted between score/exp units so the
            # engine pipeline never leaves ACT waiting on the next scores.
            fillers = list(fillers)
            qlo = kt * 128
            w = T - qlo
            pieces = [(0, min(w, 1536))]
            if w > 1536:
                pieces.append((1536, w - 1536))
            rs_t = st.tile([128, 2], F32, tag="rs", name=f"rs{duo}_{kt}")
            sums_t = (st.tile([128, 4], F32, tag="sums", name=f"sm{duo}_{kt}")
                      if len(pieces) > 1 else None)
            for hh in range(2):
                d0 = 64 * hh
                for pi, (poff, pw) in enumerate(pieces):
                    if hh + pi > 0 and fillers:
                        fillers.pop(0)()
                    sc = sc_pool.tile([128, 1536], F32, tag="sc", name="sc")
                    for co in range(0, pw, 512):
                        n = min(512, pw - co)
                        nc.tensor.matmul(
                            sc[:, co:co + n],
                            lhsT=kt_t[d0:d0 + 64, duo, qlo:qlo + 128],
                            rhs=qt_t[d0:d0 + 64, duo,
                                     qlo + poff + co:qlo + poff + co + n],
                            start=True,
                            stop=not (poff == 0 and co == 0),
                        )
                    if poff == 0:
                        nc.tensor.matmul(
                            sc[:, 0:128], lhsT=masku_t[:, 0:128],
                            rhs=ident_t[:], start=False, stop=True,
                            skip_group_check=True)
                    if kt >= 14:
                        dst = p_tail[(duo, hh, kt)][:, poff:poff + pw]
                    else:
                        p = kt // 2
                        par = kt % 2
                        off = 128 * par + poff
                        dst = p_pair[(duo, hh, p)][:, par, off:off + pw]
                    acc = (sums_t[:, hh * 2 + pi:hh * 2 + pi + 1]
                           if sums_t is not None else rs_t[:, hh:hh + 1])
                    nc.scalar.activation(
                        dst, sc[:, 0:pw], EXP,
                        bias=biasc_t[:, kt:kt + 1], scale=SCALE,
                        accum_out=acc)
            if sums_t is not None:
                for hh in range(2):
                    nc.vector.reduce_sum(
                        rs_t[:, hh:hh + 1], sums_t[:, hh * 2:hh * 2 + 2],
                        axis=AX)
            for f in fillers:
                f()
            return rs_t

        def emit_vtilde(duo, kt, rs_t):
            # rinv = 1/r~ ; V~ = 32*v*rinv (fp8 pairs / fp16 tail)
            rinv_t = st.tile([128, 2], F32, tag="rinv", name=f"ri{duo}_{kt}")
            nc.vector.reciprocal(rinv_t[:], rs_t[:])
            for hh in range(2):
                if kt >= 14:
                    dst = vp_tail[(duo, kt)][:, 64 * hh:64 * hh + 64]
                else:
                    dst = vp_pair[(duo, kt // 2)][:, kt % 2,
                                                  64 * hh:64 * hh + 64]
                nc.vector.tensor_scalar_mul(
                    dst,
                    v_t[:, kt, duo * 128 + 64 * hh:duo * 128 + 64 * hh + 64],
                    rinv_t[:, hh:hh + 1])

        def emit_av_qtile(duo, j, ot_sb):
            # O[q, d] for q-tile j: fp8 DoubleRow, P~ stationary, V~ moving
            # with both heads side-by-side; fp16 solo for key-tiles 14, 15.
            # Out partitions = queries (always base 0).
            ob = pv_tile(f"av{duo}_{j}")
            obq = ob[:, 0:128]
            plast = min(j // 2, NPAIR - 1)
            for hh in range(2):
                for p in range(plast + 1):
                    c0 = 128 * j - 256 * p
                    nc.tensor.matmul(
                        obq[:, 64 * hh:64 * hh + 64],
                        lhsT=p_pair[(duo, hh, p)][:, :, c0:c0 + 128],
                        rhs=vp_pair[(duo, p)][:, :, 64 * hh:64 * hh + 64],
                        start=(p == 0),
                        stop=(j < 14 and p == plast),
                        perf_mode=DRM,
                        skip_group_check=True,
                    )
                for kt in (14, 15):
                    if kt > j:
                        continue
                    c0 = 128 * j - 128 * kt
                    nc.tensor.matmul(
                        obq[:, 64 * hh:64 * hh + 64],
                        lhsT=p_tail[(duo, hh, kt)][:, c0:c0 + 128],
                        rhs=vp_tail[(duo, kt)][:, 64 * hh:64 * hh + 64],
                        start=False,
                        stop=(kt == min(j, 15)),
                        skip_group_check=True,
                    )
            nc.vector.tensor_scalar_mul(
                ot_sb[:, 128 * j:128 * j + 128], obq, 1.0 / 32.0)
            nc.sync.dma_start(
                ot_d.ap()[128 * j:128 * j + 128,
                          duo * 128:duo * 128 + 128],
                ot_sb[:, 128 * j:128 * j + 128])

        # ---- schedule ----
        # duo0 fillers: own Q/K chunks just-in-time, duo1 Q/K chunks and V
        # tiles pushed toward big-kt iterations (wider ACT windows)
        qk_sched = {13: [(0, False, 2)], 12: [(0, True, 2)],
                    9: [(0, False, 1)], 8: [(0, True, 1)],
                    7: [(1, False, 3)], 6: [(1, True, 3)],
                    5: [(0, False, 0)], 4: [(0, True, 0)],
                    3: [(1, False, 2)], 2: [(1, True, 2)],
                    1: [(1, False, 1), (1, True, 1)],
                    0: [(1, False, 0), (1, True, 0)]}

        # head: 256-col sub-chunks; the [1792:2048] halves unblock kt15/14,
        # the [1536:1792] halves are emitted BETWEEN kt15 and kt13 (PE runs
        # its queue in program order)
        emit_qk_chunk(0, False, 3, off=256, n=256)
        emit_qk_chunk(0, True, 3, off=256, n=256)
        head_sched = {15: [(0, False, 3, 0, 256)],
                      14: [(0, True, 3, 0, 256)]}
        qk_sched = {k: head_sched.get(k, []) + qk_sched.get(k, [])
                    for k in set(head_sched) | set(qk_sched)}

        ot_sbs = [osb.tile([128, T], F32, tag=f"osb{d}", name=f"osb{d}")
                  for d in range(2)]
        # duo0's 16 A@V q-tiles, injected early into duo1's phase A
        av0_sched = {0: (0, 1), 1: (2, 3), 2: (4, 5), 3: (6, 7),
                     4: (8, 9), 5: (10, 11), 6: (12, 13), 7: (14, 15)}

        # duo0: key-tiles DESCENDING (small score tiles first -> early exp
        # start under partial DMA); duo1: ASCENDING so its own A@V q-tiles
        # interleave as soon as their key-pairs complete (short tail).
        rs_pend = {}
        for kt in range(NKT - 1, -1, -1):
            fills = []
            for args in qk_sched.get(kt, ()):
                fills.append(lambda a=args: emit_qk_chunk(*a))

            def vfill(jj):
                emit_v_tile(jj)
                emit_vtilde(0, jj, rs_pend.pop(jj))
            jj = kt + 4
            if jj <= NKT - 1:
                fills.append(lambda j=jj: vfill(j))
            if kt == 0:
                for jj in (3, 2, 1):
                    fills.append(lambda j=jj: vfill(j))
            rs_pend[kt] = emit_scores_exp(0, kt, fills)
        vfill(0)
        rs_prev = None
        for kt in range(NKT):
            fills = []
            if kt >= 1:
                pk = kt - 1
                fills.append(
                    lambda p=pk, r=rs_prev: emit_vtilde(1, p, r))
                if pk % 2 == 1 and pk <= 13:
                    fills.append(
                        lambda p=pk: emit_av_qtile(1, p - 1, ot_sbs[1]))
                    fills.append(
                        lambda p=pk: emit_av_qtile(1, p, ot_sbs[1]))
                elif pk == 14:
                    fills.append(
                        lambda: emit_av_qtile(1, 14, ot_sbs[1]))
            for j in av0_sched.get(kt, ()):
                fills.append(lambda jj=j: emit_av_qtile(0, jj, ot_sbs[0]))
            rs_prev = emit_scores_exp(1, kt, fills)
        emit_vtilde(1, 15, rs_prev)
        emit_av_qtile(1, 15, ot_sbs[1])


def _get_module():
    if "nc" not in _CACHE:
        _CACHE["nc"] = _build_module()
    return _CACHE["nc"]


def _host_tables():
    k = np.arange(T)
    c = np.where(
        k < T - 256,
        2.0 ** np.round(0.5 * np.log2(1.031 * (T - k))),
        1.0)
    biasc = (-np.log(c)).reshape(NKT, 128).T.astype(np.float32)
    qi = np.arange(128)
    masku = np.where(qi[:, None] < qi[None, :], NEG, 0.0).astype(np_bf16)
    ident = np.eye(128, dtype=np.float32).astype(np_bf16)
    ones = np.ones((1, 128), np.float32).astype(np_bf16)
    return biasc, masku, ident, ones


def _split8(a):
    hi = a.astype(np_f8)
    lo = (a - hi.astype(np.float32)).astype(np_f8)
    return hi, lo


def _make_in_maps(X, Wq, bq, Wk, bk, Wv, bv):
    X = np.asarray(X, np.float32)
    biasc, masku, ident, ones = _host_tables()
    in_maps = []
    for core in range(8):
        b, g = divmod(core, 4)
        rows = slice(D2 * g, D2 * g + D2)
        xt = np.ascontiguousarray(X[b].T)              # [E, T]
        x8, xl = _split8(xt)
        x8 = np.ascontiguousarray(x8.reshape(8, 128, T).transpose(1, 0, 2))
        xl = np.ascontiguousarray(xl.reshape(8, 128, T).transpose(1, 0, 2))

        def wprep(Wfull):
            ws = np.asarray(Wfull)[rows].T.astype(np.float32) * WS  # [E, 256]
            return _split8(ws)

        wq8, wql = wprep(Wq)
        wk8, wkl = wprep(Wk)
        wv8, wvl = wprep(Wv)
        wqk = np.concatenate(
            [wq8[:, :128], wk8[:, :128], wql[:, :128], wkl[:, :128],
             wq8[:, 128:], wk8[:, 128:], wql[:, 128:], wkl[:, 128:]],
            axis=1)  # [E, 1024], duo-major
        wv8l = np.concatenate([wv8, wvl], axis=1)

        def dr3(w):  # [E, n] -> [128, 8, n]
            n = w.shape[1]
            return np.ascontiguousarray(
                w.reshape(8, 128, n).transpose(1, 0, 2))

        bqc = (WS * np.asarray(bq)[rows]).reshape(2, 128).T
        bkc = (WS * np.asarray(bk)[rows]).reshape(2, 128).T
        cf = np.concatenate([biasc, bqc, bkc], axis=1).astype(np.float32)
        mi = np.concatenate([masku, ident], axis=1)
        ob = np.concatenate(
            [ones, (WS * np.asarray(bv)[rows]).reshape(1, D2).astype(np_bf16)],
            axis=1)
        in_maps.append({
            "x8": x8, "xl": xl,
            "wqk": dr3(wqk), "wv8l": dr3(wv8l),
            "cf": np.ascontiguousarray(cf),
            "mi": np.ascontiguousarray(mi),
            "ob": np.ascontiguousarray(ob),
        })
    return in_maps


def kernel(X, Wq, bq, Wk, bk, Wv, bv, **kw):
    in_maps = _make_in_maps(X, Wq, bq, Wk, bk, Wv, bv)
    nc = _get_module()
    res = run_bass_kernel_spmd(nc, in_maps, core_ids=list(range(8)), **kw)
    _CACHE["last_res"] = res
    out = np.zeros((B, T, E), np.float32)
    for c in range(8):
        b, g = divmod(c, 4)
        out[b, :, D2 * g:D2 * g + D2] = res.results[c]["ot"]
    return out


if __name__ == "__main__":
    _get_module()
    print("module built ok")
